# revision 1
# baseline (speedup 1.0000x reference)
"""Multi-head self-attention (B=4, S=2048, D=2048, H=16, hd=128) on 8 trn2
NeuronCores.

Sharding: tensor-parallel over heads. Core c owns heads {2c, 2c+1}:
  - computes q/k/v projections for its 2 heads over all tokens,
  - runs causal attention for its (4 batches x 2 heads) units,
  - computes a partial output projection with its 256 rows of Wo.
Host sums the 8 partial outputs and adds bo.

On-chip layouts keep activations transposed ([feature, token]) so no
transposes are needed anywhere except V (PE-transpose per 128x128 block):
  qT/kT: [j, t] from W-stationary matmuls (lhsT = W tile, rhs = xT tile)
  S^T:   [t_k, t_q] tiles (lhsT = kT tile, rhs = qT chunk); softmax runs
         along the partition axis: exp on ACT (no max subtraction --
         weights are scaled 0.02, logits are O(1)); the exp-sum G
         accumulates in two partial chains (DVE + GPSIMD), is summed and
         broadcast across partitions by one GPSIMD partition_all_reduce,
         inverted in place (DVE reciprocal), and applied by one DVE
         multiply.
  U^T:   [j, t_q] = accumulated (lhsT = V tile [t_k, j], rhs = exp(S^T)).
  O^T:   [d', t] partial = (lhsT = Wo tile [j, d'], rhs = Yn^T).
Causality: only lower-triangle key-tiles are computed; the 4 diagonal
128x512 tile positions use precomputed 0/1 masks (multiplied after exp).

Scheduling shape (per batch): projections -> V transposes -> attention
units (h, c), with the previous unit's softmax normalization emitted at
the start of the next unit and the PREVIOUS batch's output-projection
tile-groups interleaved through the ACT-paced attention stream so the
tensor engine always has independent work.
"""

import math

import numpy as np
import ml_dtypes

import concourse.bass as bass
import concourse.bacc as bacc
import concourse.mybir as mybir
import concourse.tile as tile
from concourse.masks import make_identity
from concourse.bass_utils import run_bass_kernel_spmd

BF16 = mybir.dt.bfloat16
F32 = mybir.dt.float32

B, S, D_MODEL = 4, 2048, 2048
N_HEADS, HEAD_DIM = 16, 128
N_CORES = 8
H_PER = N_HEADS // N_CORES          # 2 heads per core
JL = H_PER * HEAD_DIM               # 256 local j-columns per of q/k/v
T = B * S                           # 8192 tokens
KD = D_MODEL // 128                 # 16 contraction tiles over d_model
TC = S // 512                       # 4 token chunks of 512 per batch
NJM = 3 * H_PER                     # 6 output j-tiles for fused qkv
SCALE = 1.0 / math.sqrt(HEAD_DIM)

_CACHED_NC = None
_OCOPY_MIX = True   # o_sb copies alternate DVE/ACT


def build_program():
    nc = bacc.Bacc("TRN2", target_bir_lowering=False, debug=False)

    xT = nc.dram_tensor("xT", [D_MODEL, T], BF16, kind="ExternalInput").ap()
    wqkv = nc.dram_tensor("wqkv", [D_MODEL, 3 * JL], BF16, kind="ExternalInput").ap()
    bqkv = nc.dram_tensor("bqkv", [3 * JL], F32, kind="ExternalInput").ap()
    wo = nc.dram_tensor("wo", [JL, D_MODEL], BF16, kind="ExternalInput").ap()
    outT = nc.dram_tensor("outT", [D_MODEL, T], F32, kind="ExternalOutput").ap()

    xT_r = xT.rearrange("(k p) t -> p k t", p=128)        # [128, KD, T]

    with tile.TileContext(nc) as tc:
        with (
            tc.tile_pool(name="const", bufs=1) as const,
            tc.tile_pool(name="work", bufs=1) as work,
            tc.tile_pool(name="psum", bufs=1, space="PSUM") as psum,
        ):
            # ---- constants ----
            # Load order matters at startup: the first matmul group only
            # needs wqkv k-chunk 0 and the first xt chunk (emitted by the
            # first _emit_batch), so everything else trails them.
            wqkv_sb = const.tile([128, KD, 3 * JL], BF16)
            wqkv_r = wqkv.rearrange("(k p) j -> p k j", p=128)
            nc.sync.dma_start(wqkv_sb[:, 0:4, :], wqkv_r[:, 0:4, :])
            bqkv_sb = const.tile([128, NJM], F32)
            nc.sync.dma_start(bqkv_sb[:], bqkv.rearrange("(m p) -> p m", p=128))

            def load_trailing_consts():
                for kc in range(1, 4):
                    nc.sync.dma_start(wqkv_sb[:, 4 * kc:4 * (kc + 1), :],
                                      wqkv_r[:, 4 * kc:4 * (kc + 1), :])
                nc.sync.dma_start(wo_sb[:], wo.rearrange("(k p) d -> p k d", p=128))
            wo_sb = const.tile([128, JL // 128, D_MODEL], BF16)

            ident = const.tile([128, 128], BF16)
            make_identity(nc, ident[:])
            ones_c = const.tile([128, 1], F32)
            nc.gpsimd.memset(ones_c[:], 1.0)

            # masks[i][r, u] = 1.0 if u >= 128*i + r else 0  (diagonal tiles)
            masks = const.tile([128, 4, 512], BF16)
            nc.gpsimd.memset(masks[:], 1.0)
            for i in range(4):
                nc.gpsimd.affine_select(
                    out=masks[:, i, :],
                    in_=masks[:, i, :],
                    compare_op=mybir.AluOpType.is_ge,
                    fill=0.0,
                    base=-128 * i,
                    pattern=[[1, 512]],
                    channel_multiplier=-1,
                )

            outproj_q = []       # deferred outproj thunks from previous batch
            for b in range(B):
                _emit_batch(nc, tc, work, psum, b,
                            xT_r, wqkv_sb, bqkv_sb, wo_sb,
                            ident, ones_c, masks, outT, outproj_q,
                            post_first_xt=load_trailing_consts if b == 0 else None)
            for thunk in outproj_q:
                thunk()

    nc.compile()
    return nc


def _emit_batch(nc, tc, work, psum, b, xT_r, wqkv_sb, bqkv_sb, wo_sb,
                ident, ones_c, masks, outT, outproj_q, post_first_xt=None):
    t0 = b * S

    # ---- q/k/v projections: qkvT[j, t] for the 6 local j-tiles ----
    qkvT = work.tile([128, NJM, S], BF16, tag="qkvT", bufs=2)
    for tcn in range(TC):
      with nc.named_scope(f"proj.b{b}.t{tcn}"):
        xt = work.tile([128, KD, 512], BF16, tag="xt", bufs=2)
        nc.sync.dma_start(xt[:], xT_r[:, :, t0 + tcn * 512: t0 + (tcn + 1) * 512])
        if post_first_xt is not None:
            post_first_xt()
            post_first_xt = None
        for jm in range(NJM):
            ps = psum.tile([128, 512], F32, tag="pp", bufs=2)
            for k in range(KD):
                nc.tensor.matmul(
                    ps[:],
                    lhsT=wqkv_sb[:, k, jm * 128:(jm + 1) * 128],
                    rhs=xt[:, k, :],
                    start=(k == 0), stop=(k == KD - 1),
                )
            nc.vector.tensor_scalar_add(
                qkvT[:, jm, tcn * 512:(tcn + 1) * 512], ps[:],
                bqkv_sb[:, jm:jm + 1],
            )

    # ---- V[t, j] per head via PE transpose of vT ----
    v_sb = work.tile([128, H_PER, S // 128, 128], BF16, tag="v", bufs=1)
    for h in range(H_PER):
      with nc.named_scope(f"vtr.b{b}.h{h}"):
        for m in range(S // 128):
            vt_ps = psum.tile([128, 128], BF16, tag="pp", bufs=2)
            nc.tensor.transpose(
                vt_ps[:], qkvT[:, 2 * H_PER + h, m * 128:(m + 1) * 128], ident[:]
            )
            nc.vector.tensor_copy(v_sb[:, h, m, :], vt_ps[:])

    # ---- attention ----
    yn = work.tile([128, H_PER, S], BF16, tag="yn", bufs=2)
    pending = None

    def emit_norm(p):
      with nc.named_scope(f"norm.b{b}"):
        gs_, u_, h_, c_ = p
        if len(gs_) > 1:
            nc.vector.tensor_add(gs_[0][:], gs_[0][:], gs_[1][:])
        import concourse.bass_isa as bass_isa
        rb_sb = work.tile([128, 512], F32, tag="rb", bufs=2)
        nc.gpsimd.partition_all_reduce(rb_sb[:], gs_[0][:], channels=128,
                                       reduce_op=bass_isa.ReduceOp.add)
        nc.vector.reciprocal(rb_sb[:], rb_sb[:])
        nc.vector.tensor_mul(
            yn[:, h_, c_ * 512:(c_ + 1) * 512], u_[:], rb_sb[:]
        )

    n_units = H_PER * TC
    per_unit = (len(outproj_q) + n_units - 1) // n_units if outproj_q else 0
    unit_idx = 0
    for h in range(H_PER):
        qT = qkvT[:, h, :]
        kT = qkvT[:, H_PER + h, :]
        for c in range(TC):
            nm = 4 * (c + 1)            # valid 128-wide key tiles
            # normalization of the previous unit goes first so its pool/DVE
            # ops are not stuck behind this unit's accumulation chain
            if pending is not None:
                emit_norm(pending)
                pending = None
            # The exp-sum G is accumulated in two independent partial chains
            # (DVE 2/3 of pairs, GPSIMD 1/3) so neither engine's serial chain
            # outlasts the unit; the norm that consumes them is deferred by
            # one unit, and sums both partials into one PSUM accumulator.
            with nc.named_scope(f"att.b{b}.u{unit_idx}"):
              g_d = g_p = None
              e_pairs = []
              npr = nm // 2
              # spread this unit's share of deferred outproj groups through
              # the ACT-paced pair loop so PE always has independent work
              spots = set(np.linspace(0, npr - 1, min(per_unit, npr)).astype(int).tolist()) if outproj_q else set()
              popped = 0
              for pr in range(nm // 2):
                  if pr in spots and outproj_q:
                      outproj_q.pop(0)()
                      popped += 1
                  s2 = psum.tile([128, 2, 512], F32, tag="s2", bufs=2)
                  for i in range(2):
                      m = 2 * pr + i
                      nc.tensor.matmul(
                          s2[:, i, :],
                          lhsT=kT[:, m * 128:(m + 1) * 128],
                          rhs=qT[:, c * 512:(c + 1) * 512],
                          start=True, stop=True,
                      )
                  e = work.tile([128, 2, 512], BF16, tag="e", bufs=9)
                  nc.scalar.activation(e[:], s2[:], mybir.ActivationFunctionType.Exp,
                                       scale=SCALE)
                  if pr >= nm // 2 - 2:   # diagonal pairs get the causal mask
                      i0 = 2 * (pr - (nm // 2 - 2))
                      nc.vector.tensor_mul(e[:], e[:], masks[:, i0:i0 + 2, :])
                  if pr % 3 == 2:
                      if g_p is None:
                          g_p = work.tile([128, 512], F32, tag="gp", bufs=2)
                          nc.gpsimd.tensor_add(g_p[:], e[:, 0, :], e[:, 1, :])
                      else:
                          nc.gpsimd.tensor_add(g_p[:], g_p[:], e[:, 0, :])
                          nc.gpsimd.tensor_add(g_p[:], g_p[:], e[:, 1, :])
                  else:
                      if g_d is None:
                          g_d = work.tile([128, 512], F32, tag="g", bufs=2)
                          nc.vector.tensor_add(g_d[:], e[:, 0, :], e[:, 1, :])
                      else:
                          nc.vector.tensor_add(g_d[:], g_d[:], e[:, 0, :])
                          nc.vector.tensor_add(g_d[:], g_d[:], e[:, 1, :])
                  e_pairs.append(e)

              u = psum.tile([128, 512], F32, tag="u", bufs=2)
              for m in range(nm):
                  nc.tensor.matmul(
                      u[:],
                      lhsT=v_sb[:, h, m, :],
                      rhs=e_pairs[m // 2][:, m % 2, :],
                      start=(m == 0), stop=(m == nm - 1),
                  )
              pending = ([g for g in (g_d, g_p) if g is not None], u, h, c)
            # remainder of this unit's share of deferred outproj groups
            for _ in range(per_unit - popped):
                if outproj_q:
                    outproj_q.pop(0)()
            unit_idx += 1
    emit_norm(pending)
    while outproj_q:
        outproj_q.pop(0)()

    # ---- partial output projection (deferred into the next batch) ----
    def make_outproj(dm, tcn, yn=yn, t0=t0):
        def thunk():
          with nc.named_scope(f"oproj.b{b}"):
            ps = psum.tile([128, 512], F32, tag="pp", bufs=2)
            for kj in range(JL // 128):
                nc.tensor.matmul(
                    ps[:],
                    lhsT=wo_sb[:, kj, dm * 128:(dm + 1) * 128],
                    rhs=yn[:, kj, tcn * 512:(tcn + 1) * 512],
                    start=(kj == 0), stop=(kj == JL // 128 - 1),
                )
            o_sb = work.tile([128, 512], F32, tag="osb", bufs=3)
            if _OCOPY_MIX and (dm * TC + tcn) % 2 == 1:
                nc.scalar.copy(o_sb[:], ps[:])
            else:
                nc.vector.tensor_copy(o_sb[:], ps[:])
            nc.sync.dma_start(
                outT[dm * 128:(dm + 1) * 128,
                     t0 + tcn * 512: t0 + (tcn + 1) * 512],
                o_sb[:],
            )
        return thunk

    for dm in range(D_MODEL // 128):
        for tcn in range(TC):
            outproj_q.append(make_outproj(dm, tcn))


def make_in_maps(x, Wq, bq, Wk, bk, Wv, bv, Wo, bo):
    xT_np = np.ascontiguousarray(
        x.reshape(T, D_MODEL).T).astype(ml_dtypes.bfloat16)
    in_maps = []
    for c in range(N_CORES):
        sl = slice(c * JL, (c + 1) * JL)
        wqkv_np = np.concatenate(
            [Wq[:, sl], Wk[:, sl], Wv[:, sl]], axis=1).astype(ml_dtypes.bfloat16)
        bqkv_np = np.concatenate([bq[sl], bk[sl], bv[sl]]).astype(np.float32)
        wo_np = np.ascontiguousarray(Wo[sl, :]).astype(ml_dtypes.bfloat16)
        in_maps.append({
            "xT": xT_np, "wqkv": wqkv_np, "bqkv": bqkv_np, "wo": wo_np,
        })
    return in_maps


def kernel(x, Wq, bq, Wk, bk, Wv, bv, Wo, bo):
    global _CACHED_NC
    x, Wq, bq, Wk, bk, Wv, bv, Wo, bo = [
        np.asarray(a, np.float32) for a in (x, Wq, bq, Wk, bk, Wv, bv, Wo, bo)
    ]
    if _CACHED_NC is None:
        _CACHED_NC = build_program()
    nc = _CACHED_NC

    in_maps = make_in_maps(x, Wq, bq, Wk, bk, Wv, bv, Wo, bo)
    res = run_bass_kernel_spmd(nc, in_maps, core_ids=list(range(N_CORES)))

    acc = res.results[0]["outT"].astype(np.float32)
    for c in range(1, N_CORES):
        acc += res.results[c]["outT"]
    out = acc.T + bo[None, :]
    return np.ascontiguousarray(out.reshape(B, S, D_MODEL), dtype=np.float32)


# ---------------------------------------------------------------- dev tools

def _np_partial_reference(inputs, core):
    """fp32 numpy partial output for one core's heads (no bo)."""
    x = np.asarray(inputs["x"], np.float32).reshape(T, D_MODEL)
    sl = slice(core * JL, (core + 1) * JL)
    q = x @ np.asarray(inputs["Wq"])[:, sl] + np.asarray(inputs["bq"])[sl]
    k = x @ np.asarray(inputs["Wk"])[:, sl] + np.asarray(inputs["bk"])[sl]
    v = x @ np.asarray(inputs["Wv"])[:, sl] + np.asarray(inputs["bv"])[sl]
    y = np.zeros((T, JL), np.float32)
    for b in range(B):
        tb = slice(b * S, (b + 1) * S)
        for h in range(H_PER):
            js = slice(h * HEAD_DIM, (h + 1) * HEAD_DIM)
            qh, kh, vh = q[tb, js], k[tb, js], v[tb, js]
            s = (qh @ kh.T) * SCALE
            mask = np.triu(np.ones((S, S), bool), k=1)
            s[mask] = -np.inf
            s -= s.max(axis=1, keepdims=True)
            p = np.exp(s)
            p /= p.sum(axis=1, keepdims=True)
            y[tb, js] = p @ vh
    return (y @ np.asarray(inputs["Wo"])[sl, :]).T  # [D, T]


def _simulate_core0():
    import reference
    from concourse.bass_interp import CoreSim

    inputs = {k: np.asarray(v) for k, v in reference.setup_inputs().items()}
    nc = build_program()
    in_map = make_in_maps(**inputs)[0]

    sim = CoreSim(nc)
    for name, arr in in_map.items():
        sim.tensor(name)[:] = arr
    sim.simulate(check_with_hw=False)
    got = np.asarray(sim.tensor("outT"), np.float32)

    want = _np_partial_reference(inputs, 0)
    denom = np.abs(want).max()
    err = np.abs(got - want).max() / denom
    print(f"sim core0 partial: max={np.abs(got).max():.4f} "
          f"absmax_err={np.abs(got - want).max():.5f} rel={err:.5f}")


if __name__ == "__main__":
    import sys
    if "--sim" in sys.argv:
        _simulate_core0()
    else:
        nc = build_program()
        n_inst = sum(len(bb.instructions) for bb in nc.m.functions[0].blocks)
        print(f"built: {n_inst} instructions")



# revision 4
# speedup vs baseline: 1.1478x; 1.1478x over previous
"""Multi-head self-attention (B=4, S=2048, D=2048, H=16, hd=128) on 8 trn2
NeuronCores.

Sharding: tensor-parallel over heads. Core c owns heads {2c, 2c+1}:
  - computes q/k/v projections for its 2 heads over all tokens,
  - runs causal attention for its (4 batches x 2 heads) units,
  - computes a partial output projection with its 256 rows of Wo.
Host sums the 8 partial outputs (fp16) and adds bo.

On-chip layouts:
  qT/kT: [j, t] from W-stationary matmuls (lhsT = W tile, rhs = xT tile).
  V:     [t, j] computed DIRECTLY via swapped matmul (lhsT = xT token tile,
         rhs = Wv chunk) -- no PE transposes needed at all.
  S^T:   [t_k, t_q] tiles (lhsT = kT tile, rhs = qT chunk); softmax runs
         along the partition axis: exp on ACT (no max subtraction --
         weights are scaled 0.02, logits are O(1)); the exp-sum G
         accumulates as fp16 pair-sums (Pool/DVE alternating) chained into
         one fp16 accumulator (DVE 2x mode), reduced across partitions by
         one GPSIMD partition_all_reduce (f32 out), inverted (DVE
         reciprocal) and applied by one DVE multiply.
  U^T:   [j, t_q] = accumulated (lhsT = V tile [t_k, j], rhs = exp(S^T)).
  O^T:   [d', t] partial = (lhsT = Wo tile [j, d'], rhs = Yn^T), written
         to DRAM as fp16 to halve output DMA traffic.
Causality: only lower-triangle key-tiles are computed; the 4 diagonal
128x512 tile positions use precomputed 0/1 masks (multiplied after exp).

Scheduling shape (per batch): projections (q,k j-tiles + v token-tiles
per 512-chunk) -> attention units in (chunk, head) order. Output
projection tile-groups for chunk c are queued once both heads' yn for c
is normalized and popped into later units' ACT-paced pair loops (and into
the next batch's projection phase), so the tensor engine always has
independent work and no large drain remains at the end. PSUM->SBUF
copies rotate across Pool/ACT/DVE; input DMAs ride the ACT queue,
output DMAs the sync queue.
"""

import math

import numpy as np
import ml_dtypes

import concourse.bass as bass
import concourse.bacc as bacc
import concourse.mybir as mybir
import concourse.tile as tile
from concourse.bass_utils import run_bass_kernel_spmd

BF16 = mybir.dt.bfloat16
F16 = mybir.dt.float16
F32 = mybir.dt.float32

B, S, D_MODEL = 4, 2048, 2048
N_HEADS, HEAD_DIM = 16, 128
N_CORES = 8
H_PER = N_HEADS // N_CORES          # 2 heads per core
JL = H_PER * HEAD_DIM               # 256 local j-columns per of q/k/v
T = B * S                           # 8192 tokens
KD = D_MODEL // 128                 # 16 contraction tiles over d_model
TC = S // 512                       # 4 token chunks of 512 per batch
SCALE = 1.0 / math.sqrt(HEAD_DIM)

_CACHED_NC = None


def build_program():
    nc = bacc.Bacc("TRN2", target_bir_lowering=False, debug=False)

    xT = nc.dram_tensor("xT", [D_MODEL, T], BF16, kind="ExternalInput").ap()
    wqkv = nc.dram_tensor("wqkv", [D_MODEL, 3 * JL], BF16, kind="ExternalInput").ap()
    bqk = nc.dram_tensor("bqk", [128, 4], F32, kind="ExternalInput").ap()
    bvb = nc.dram_tensor("bvb", [128, JL], F32, kind="ExternalInput").ap()
    wo = nc.dram_tensor("wo", [JL, D_MODEL], BF16, kind="ExternalInput").ap()
    outT = nc.dram_tensor("outT", [D_MODEL, T], F16, kind="ExternalOutput").ap()

    xT_r = xT.rearrange("(k p) t -> p k t", p=128)        # [128, KD, T]

    with tile.TileContext(nc) as tc:
        with (
            nc.allow_low_precision(reason="fp16 G accumulation is plenty"),
            tc.tile_pool(name="const", bufs=1) as const,
            tc.tile_pool(name="work", bufs=1) as work,
            tc.tile_pool(name="psum", bufs=1, space="PSUM") as psum,
        ):
            # ---- constants ----
            # Load order matters at startup: the first matmul group only
            # needs wqkv k-chunk 0 and the first xt chunk, so everything
            # else trails them.
            wqkv_sb = const.tile([128, KD, 3 * JL], BF16)
            wqkv_r = wqkv.rearrange("(k p) j -> p k j", p=128)
            nc.sync.dma_start(wqkv_sb[:, 0:4, :], wqkv_r[:, 0:4, :])
            bqk_sb = const.tile([128, 4], F32)
            nc.sync.dma_start(bqk_sb[:], bqk)
            bvb_sb = const.tile([128, JL], F32)
            nc.sync.dma_start(bvb_sb[:], bvb)

            def load_trailing_consts():
                for kc in range(1, 4):
                    nc.sync.dma_start(wqkv_sb[:, 4 * kc:4 * (kc + 1), :],
                                      wqkv_r[:, 4 * kc:4 * (kc + 1), :])
                nc.sync.dma_start(wo_sb[:], wo.rearrange("(k p) d -> p k d", p=128))
            wo_sb = const.tile([128, JL // 128, D_MODEL], BF16)

            st = {"opq": [], "copy_rr": 0, "pair_rr": 0}
            for b in range(B):
                _emit_batch(nc, tc, work, psum, b,
                            xT_r, wqkv_sb, bqk_sb, bvb_sb, wo_sb,
                            outT, st,
                            post_first_xt=load_trailing_consts if b == 0 else None)
            while st["opq"]:
                st["opq"].pop(0)()

    nc.compile()
    return nc


def _emit_batch(nc, tc, work, psum, b, xT_r, wqkv_sb, bqk_sb, bvb_sb, wo_sb,
                outT, st, post_first_xt=None):
    t0 = b * S

    def pop_op(n=1):
        for _ in range(n):
            if st["opq"]:
                st["opq"].pop(0)()

    # ---- projections ----
    # qkT[j, t] for the 4 local q/k j-tiles; v_sb[t, j] token tiles direct.
    qkT = work.tile([128, 2 * H_PER, S], BF16, tag="qkT", bufs=2)
    v_sb = work.tile([128, S // 128, JL], BF16, tag="v", bufs=2)
    for tcn in range(TC):
      with nc.named_scope(f"proj.b{b}.t{tcn}"):
        xt = work.tile([128, KD, 512], BF16, tag="xt", bufs=2)
        for piece in range(4):
            nc.scalar.dma_start(
                xt[:, 4 * piece:4 * (piece + 1), :],
                xT_r[:, 4 * piece:4 * (piece + 1),
                     t0 + tcn * 512: t0 + (tcn + 1) * 512])
        if post_first_xt is not None:
            post_first_xt()
            post_first_xt = None
        for jm in range(2 * H_PER):      # q0 q1 k0 k1
            pop_op()
            ps = psum.tile([128, 512], F32, tag="pp", bufs=2)
            for k in range(KD):
                nc.tensor.matmul(
                    ps[:],
                    lhsT=wqkv_sb[:, k, jm * 128:(jm + 1) * 128],
                    rhs=xt[:, k, :],
                    start=(k == 0), stop=(k == KD - 1),
                )
            nc.vector.tensor_scalar_add(
                qkT[:, jm, tcn * 512:(tcn + 1) * 512], ps[:],
                bqk_sb[:, jm:jm + 1],
            )
        for vt in range(4):              # v token-tiles, [128t, 256j]
            pop_op()
            ps = psum.tile([128, 512], F32, tag="pp", bufs=2)
            for k in range(KD):
                nc.tensor.matmul(
                    ps[:, 0:JL],
                    lhsT=xt[:, k, vt * 128:(vt + 1) * 128],
                    rhs=wqkv_sb[:, k, 2 * JL:3 * JL],
                    start=(k == 0), stop=(k == KD - 1),
                )
            nc.vector.tensor_add(v_sb[:, 4 * tcn + vt, :], ps[:, 0:JL],
                                 bvb_sb[:])

    # ---- attention ----
    yn = work.tile([128, H_PER, S], BF16, tag="yn", bufs=2)
    pending = None

    def emit_norm(p):
      with nc.named_scope(f"norm.b{b}"):
        g_, u_, h_, c_ = p
        import concourse.bass_isa as bass_isa
        rb_sb = work.tile([128, 512], F32, tag="rb", bufs=2)
        nc.gpsimd.partition_all_reduce(rb_sb[:], g_[:], channels=128,
                                       reduce_op=bass_isa.ReduceOp.add)
        nc.vector.reciprocal(rb_sb[:], rb_sb[:])
        nc.vector.tensor_mul(
            yn[:, h_, c_ * 512:(c_ + 1) * 512], u_[:], rb_sb[:]
        )

    for c in range(TC):
        for h in range(H_PER):
            nm = 4 * (c + 1)            # valid 128-wide key tiles
            npr = nm // 2
            if pending is not None:
                emit_norm(pending)
                pending = None
            with nc.named_scope(f"att.b{b}.c{c}.h{h}"):
              qT = qkT[:, h, :]
              kT = qkT[:, H_PER + h, :]
              g = None
              e_pairs = []
              for pr in range(npr):
                  pop_op(2 if pr else 1)
                  s2 = psum.tile([128, 2, 512], F32, tag="s2", bufs=2)
                  for i in range(2):
                      m = 2 * pr + i
                      nc.tensor.matmul(
                          s2[:, i, :],
                          lhsT=kT[:, m * 128:(m + 1) * 128],
                          rhs=qT[:, c * 512:(c + 1) * 512],
                          start=True, stop=True,
                      )
                  e = work.tile([128, 2, 512], BF16, tag="e", bufs=8)
                  nc.scalar.activation(e[:], s2[:], mybir.ActivationFunctionType.Exp,
                                       scale=SCALE)
                  if pr >= npr - 2:   # diagonal pairs get the causal mask
                      i0 = 2 * (pr - (npr - 2))
                      for i in range(2):
                          nc.gpsimd.affine_select(
                              out=e[:, i, :], in_=e[:, i, :],
                              compare_op=mybir.AluOpType.is_ge,
                              fill=0.0, base=-128 * (i0 + i),
                              pattern=[[1, 512]], channel_multiplier=-1)
                  # G accumulation: pair-sum on Pool/DVE alternating, then
                  # one serial fp16 chain add on DVE (2x mode).
                  if g is None:
                      g = work.tile([128, 512], F16, tag="g", bufs=2)
                      nc.vector.tensor_add(g[:], e[:, 0, :], e[:, 1, :])
                  else:
                      ph = work.tile([128, 512], F16, tag="ph", bufs=3)
                      nc.gpsimd.tensor_add(ph[:], e[:, 0, :], e[:, 1, :])
                      nc.vector.tensor_add(g[:], g[:], ph[:])
                  e_pairs.append(e)

              u = psum.tile([128, 512], F32, tag="u", bufs=2)
              for m in range(nm):
                  nc.tensor.matmul(
                      u[:],
                      lhsT=v_sb[:, m, h * 128:(h + 1) * 128],
                      rhs=e_pairs[m // 2][:, m % 2, :],
                      start=(m == 0), stop=(m == nm - 1),
                  )
              pending = (g, u, h, c)
        # both heads of chunk c are now (pending-)normalized; queue its
        # output projection tile-groups.
        for dm in range(D_MODEL // 128):
            st["opq"].append(_make_outproj(nc, work, psum, wo_sb, outT, st,
                                           b, dm, c, yn, t0))
    emit_norm(pending)


def _make_outproj(nc, work, psum, wo_sb, outT, st, b, dm, tcn, yn, t0):
    def thunk():
      with nc.named_scope(f"oproj.b{b}"):
        ps = psum.tile([128, 512], F32, tag="pp", bufs=2)
        for kj in range(JL // 128):
            nc.tensor.matmul(
                ps[:],
                lhsT=wo_sb[:, kj, dm * 128:(dm + 1) * 128],
                rhs=yn[:, kj, tcn * 512:(tcn + 1) * 512],
                start=(kj == 0), stop=(kj == JL // 128 - 1),
            )
        o_sb = work.tile([128, 512], F16, tag="osb", bufs=4)
        rr = st["copy_rr"] % 2
        st["copy_rr"] += 1
        if rr == 0:
            nc.scalar.copy(o_sb[:], ps[:])
        else:
            nc.vector.tensor_copy(o_sb[:], ps[:])
        nc.sync.dma_start(
            outT[dm * 128:(dm + 1) * 128,
                 t0 + tcn * 512: t0 + (tcn + 1) * 512],
            o_sb[:],
        )
    return thunk


def make_in_maps(x, Wq, bq, Wk, bk, Wv, bv, Wo, bo):
    xT_np = np.ascontiguousarray(
        x.reshape(T, D_MODEL).T).astype(ml_dtypes.bfloat16)
    in_maps = []
    for c in range(N_CORES):
        sl = slice(c * JL, (c + 1) * JL)
        wqkv_np = np.concatenate(
            [Wq[:, sl], Wk[:, sl], Wv[:, sl]], axis=1).astype(ml_dtypes.bfloat16)
        bqk_np = np.concatenate([bq[sl], bk[sl]]).astype(np.float32)
        bqk_np = np.ascontiguousarray(bqk_np.reshape(4, 128).T)
        bvb_np = np.ascontiguousarray(
            np.broadcast_to(bv[sl].astype(np.float32)[None, :], (128, JL)))
        wo_np = np.ascontiguousarray(Wo[sl, :]).astype(ml_dtypes.bfloat16)
        in_maps.append({
            "xT": xT_np, "wqkv": wqkv_np, "bqk": bqk_np, "bvb": bvb_np,
            "wo": wo_np,
        })
    return in_maps


def kernel(x, Wq, bq, Wk, bk, Wv, bv, Wo, bo):
    global _CACHED_NC
    x, Wq, bq, Wk, bk, Wv, bv, Wo, bo = [
        np.asarray(a, np.float32) for a in (x, Wq, bq, Wk, bk, Wv, bv, Wo, bo)
    ]
    if _CACHED_NC is None:
        _CACHED_NC = build_program()
    nc = _CACHED_NC

    in_maps = make_in_maps(x, Wq, bq, Wk, bk, Wv, bv, Wo, bo)
    res = run_bass_kernel_spmd(nc, in_maps, core_ids=list(range(N_CORES)))

    acc = res.results[0]["outT"].astype(np.float32)
    for c in range(1, N_CORES):
        acc += res.results[c]["outT"].astype(np.float32)
    out = acc.T + bo[None, :]
    return np.ascontiguousarray(out.reshape(B, S, D_MODEL), dtype=np.float32)


# ---------------------------------------------------------------- dev tools

def _np_partial_reference(inputs, core):
    """fp32 numpy partial output for one core's heads (no bo)."""
    x = np.asarray(inputs["x"], np.float32).reshape(T, D_MODEL)
    sl = slice(core * JL, (core + 1) * JL)
    q = x @ np.asarray(inputs["Wq"])[:, sl] + np.asarray(inputs["bq"])[sl]
    k = x @ np.asarray(inputs["Wk"])[:, sl] + np.asarray(inputs["bk"])[sl]
    v = x @ np.asarray(inputs["Wv"])[:, sl] + np.asarray(inputs["bv"])[sl]
    y = np.zeros((T, JL), np.float32)
    for b in range(B):
        tb = slice(b * S, (b + 1) * S)
        for h in range(H_PER):
            js = slice(h * HEAD_DIM, (h + 1) * HEAD_DIM)
            qh, kh, vh = q[tb, js], k[tb, js], v[tb, js]
            s = (qh @ kh.T) * SCALE
            mask = np.triu(np.ones((S, S), bool), k=1)
            s[mask] = -np.inf
            s -= s.max(axis=1, keepdims=True)
            p = np.exp(s)
            p /= p.sum(axis=1, keepdims=True)
            y[tb, js] = p @ vh
    return (y @ np.asarray(inputs["Wo"])[sl, :]).T  # [D, T]


def _simulate_core0():
    import reference
    from concourse.bass_interp import CoreSim

    inputs = {k: np.asarray(v) for k, v in reference.setup_inputs().items()}
    nc = build_program()
    in_map = make_in_maps(**inputs)[0]

    sim = CoreSim(nc)
    for name, arr in in_map.items():
        sim.tensor(name)[:] = arr
    sim.simulate(check_with_hw=False)
    got = np.asarray(sim.tensor("outT"), np.float32)

    want = _np_partial_reference(inputs, 0)
    denom = np.abs(want).max()
    err = np.abs(got - want).max() / denom
    print(f"sim core0 partial: max={np.abs(got).max():.4f} "
          f"absmax_err={np.abs(got - want).max():.5f} rel={err:.5f}")


if __name__ == "__main__":
    import sys
    if "--sim" in sys.argv:
        _simulate_core0()
    else:
        nc = build_program()
        n_inst = sum(len(bb.instructions) for bb in nc.m.functions[0].blocks)
        print(f"built: {n_inst} instructions")
